# revision 3
# baseline (speedup 1.0000x reference)
"""Trainium2 Bass kernel for nn_CIN (3-layer CIN / xDeepFM feature-interaction).

Reference computation per layer k (x: (B,39,16), h0 = x):
    z[b,f,g,d] = x[b,f,d] * h[b,g,d]
    cur[b,l,d] = relu(sum_{f,g} z[b,f,g,d] * Wk[f*Fk+g, l] + bk[l])
    h <- cur[:, :64] (layers 0,1);  direct outputs concat'd, summed over d.

Sharding: pure data parallelism, batch 1024 -> 8 cores x 128 rows.

Device layout per core: everything is (partition, n) with n = b*16+d in [0,2048).
Per K-chunk of 128 (f,g)-pairs:
  mult path:   PE:  bc = S_c^T @ xT        (broadcast x_f to its (f,g) rows, PSUM)
               DVE: z = bc * h_rep         (outer-product tile, -> SBUF)
  square path: PE:  P = Ssq_c^T @ [xT; h]  (x_f - h_g per row, PSUM)
               ACT: z = Square(P)          (-> SBUF;  x*h = -(1/2)(P^2 - x^2 - h^2),
                                            signs/corrections folded into weights)
  both:        PE:  cur += Wc_c^T @ z      (accumulating matmul, PSUM)
Square-path chunk residuals sum_f .5*W~x[f,l]*x_f^2 + sum_g .5*W~h[g,l]*h_g^2 are
extra small K-chunks with host-folded weights against Square([xT; h]).

The split of chunks between DVE (mult) and ACT (square) balances the two
elementwise engines; TensorE runs both paths' matmuls.
"""

import numpy as np

B, F, D, L = 1024, 39, 16, 128
NCORES = 8
BC = B // NCORES          # 128 batch rows per core
NF = BC * D               # 2048 free elements per core
HALF = NF // 2            # 1024: psum-bank-pair granule
FK = (39, 64, 64)         # h rows per layer
KP = (117, 128, 128)      # chunk partition height per layer
NK = (13, 20, 20)         # chunks per layer
FPER = (3, 2, 2)          # f values per chunk

# Chunks routed to the ACT/square path, per layer (tuned on HW profile).
SQ = (
    frozenset(range(0, 13, 2)),   # 7 of 13
    frozenset(range(0, 20, 2)),   # 10 of 20
    frozenset(range(0, 20, 2)),   # 10 of 20
)

_CACHE = {}


def _f_of(layer, ci, p):
    """f,g indices of partition p within chunk ci of a layer; f may be >=39 (pad)."""
    fk = FK[layer]
    return FPER[layer] * ci + p // fk, p % fk


def _host_consts(W0, W1, W2):
    """Fold reference weights into the device constant tensors."""
    Ws = (W0.reshape(39, 39, L), W1.reshape(39, 64, L), W2.reshape(39, 64, L))
    out = {}

    # Broadcast selectors. Smul: (39, nk*kp). Ssq: (39+fk, nk*kp) for sq chunks.
    for layer in (0, 1, 2):
        if layer == 2:
            break  # layers 1,2 share selectors
        lays = (0,) if layer == 0 else (1, 2)
        kp, nk, fk = KP[layer], NK[layer], FK[layer]
        smul = np.zeros((39, nk * kp), np.float32)
        ssq = np.zeros((39 + (0 if layer == 0 else fk), nk * kp), np.float32)
        for ci in range(nk):
            for p in range(kp):
                f, g = _f_of(layer, ci, p)
                if f >= 39:
                    continue
                smul[f, ci * kp + p] = 1.0
                ssq[f, ci * kp + p] = 1.0
                grow = g if layer == 0 else 39 + g
                ssq[grow, ci * kp + p] -= 1.0
        out[f"Smul{layer}"] = smul
        out[f"Ssq{layer}"] = ssq

    # Weight chunks (sq chunks scaled by -1/2) + correction matrices.
    for layer in (0, 1, 2):
        kp, nk, fk = KP[layer], NK[layer], FK[layer]
        W = Ws[layer]
        wc = np.zeros((kp, nk * L), np.float32)
        corr = np.zeros((39 + (fk if layer > 0 else 0), L), np.float32)
        for ci in range(nk):
            sq = ci in SQ[layer]
            for p in range(kp):
                f, g = _f_of(layer, ci, p)
                if f >= 39:
                    continue
                wrow = W[f, g]
                wc[p, ci * L : (ci + 1) * L] = (-0.5 * wrow) if sq else wrow
                if sq:
                    # x*h = .5*x^2 + .5*h^2 - .5*(x-h)^2
                    corr[f] += 0.5 * wrow
                    if layer == 0:
                        corr[g] += 0.5 * wrow
                    else:
                        corr[39 + g] += 0.5 * wrow
        out[f"Wc{layer}"] = wc
        out[f"corr{layer}"] = corr
    return out


def _build_nc():
    import concourse.bacc as bacc
    import concourse.tile as tile
    from concourse import mybir

    F32 = mybir.dt.float32
    nc = bacc.Bacc("TRN2", target_bir_lowering=False, debug=False, num_devices=NCORES)

    dram = {}

    def din(name, shape):
        dram[name] = nc.dram_tensor(name, shape, F32, kind="ExternalInput").ap()

    din("xT", (39, NF))
    din("xT3", (117, NF))
    din("Smul0", (39, NK[0] * KP[0]))
    din("Smul1", (39, NK[1] * KP[1]))
    din("Ssq0", (39, NK[0] * KP[0]))
    din("Ssq1", (103, NK[1] * KP[1]))
    din("Wc0", (KP[0], NK[0] * L))
    din("Wc1", (KP[1], NK[1] * L))
    din("Wc2", (KP[2], NK[2] * L))
    din("corr0", (39, L))
    din("corr1", (103, L))
    din("corr2", (103, L))
    din("bias", (L, 3))
    out_d = nc.dram_tensor("out", (256, BC), F32, kind="ExternalOutput").ap()

    with tile.TileContext(nc) as tc:
        with (
            tc.tile_pool(name="const", bufs=1) as cp,
            tc.tile_pool(name="work", bufs=2) as wp,
            tc.tile_pool(name="relu", bufs=1) as rp,
            tc.tile_pool(name="zp", bufs=4) as zp,
            tc.tile_pool(name="pbc", bufs=2, space="PSUM") as pbc,
            tc.tile_pool(name="pcur", bufs=1, space="PSUM") as pcur,
        ):
            ct = {}
            for name in dram:
                if name == "out":
                    continue
                shape = list(dram[name].shape)
                ct[name] = cp.tile(shape, F32, tag=name, name=f"c_{name}")
                nc.sync.dma_start(out=ct[name], in_=dram[name])

            relu_t = [None] * 3
            red_t = [None] * 3

            xh = [None] * 3  # square-path rhs per layer ([xT] / [xT; h])
            xhsq = [None] * 3  # Square(xh)
            xh[0] = ct["xT"]

            for layer in (0, 1, 2):
                kp, nk, fk = KP[layer], NK[layer], FK[layer]
                smul = ct["Smul0"] if layer == 0 else ct["Smul1"]
                ssq = ct["Ssq0"] if layer == 0 else ct["Ssq1"]
                wc = ct[f"Wc{layer}"]
                sq_rows = 39 if layer == 0 else 103
                use_sq = len(SQ[layer]) > 0

                # h_rep: (kp, NF) replicated h tile
                if layer == 0:
                    h_rep = ct["xT3"]
                else:
                    prev = relu_t[layer - 1]
                    h_rep = wp.tile([128, NF], F32, tag="h_rep")
                    nc.sync.dma_start(out=h_rep[0:64, :], in_=prev[0:64, :])
                    nc.sync.dma_start(out=h_rep[64:128, :], in_=prev[0:64, :])
                    if use_sq:
                        xh[layer] = wp.tile([103, NF], F32, tag="xh", name=f"xh{layer}")
                        nc.sync.dma_start(out=xh[layer][0:39, :], in_=dram["xT"])
                        nc.sync.dma_start(
                            out=xh[layer][39:103, :], in_=prev[0:64, :]
                        )

                if use_sq:
                    xhsq[layer] = wp.tile([sq_rows, NF], F32, tag="xhsq", name=f"xhsq{layer}")
                    nc.scalar.activation(
                        out=xhsq[layer][:, :],
                        in_=xh[layer][0:sq_rows, :],
                        func=mybir.ActivationFunctionType.Square,
                    )

                cur = pcur.tile([128, NF], F32, tag="cur")
                for ci in range(nk):
                    issq = ci in SQ[layer]
                    for half in range(2):
                        ns = slice(half * HALF, (half + 1) * HALF)
                        bc = pbc.tile([kp, HALF], F32, tag="bc")
                        sel = ssq if issq else smul
                        rhs_src = xh[layer] if issq else ct["xT"]
                        nrows = sq_rows if issq else 39
                        for q in range(2):
                            qs = slice(q * 512, (q + 1) * 512)
                            nqs = slice(half * HALF + q * 512, half * HALF + (q + 1) * 512)
                            nc.tensor.matmul(
                                bc[:, qs],
                                lhsT=sel[0:nrows, ci * kp : (ci + 1) * kp],
                                rhs=rhs_src[0:nrows, nqs],
                                start=True,
                                stop=True,
                            )
                        zt = zp.tile([kp, HALF], F32, tag="z")
                        if issq:
                            nc.scalar.activation(
                                out=zt[:, :],
                                in_=bc[:, :],
                                func=mybir.ActivationFunctionType.Square,
                            )
                        else:
                            nc.vector.tensor_mul(zt[:, :], bc[:, :], h_rep[0:kp, ns])
                        for q in range(2):
                            qs = slice(q * 512, (q + 1) * 512)
                            nqs = slice(half * HALF + q * 512, half * HALF + (q + 1) * 512)
                            nc.tensor.matmul(
                                cur[:, nqs],
                                lhsT=wc[:, ci * L : (ci + 1) * L],
                                rhs=zt[:, qs],
                                start=(ci == 0),
                                stop=(ci == nk - 1 and not use_sq),
                            )

                if use_sq:
                    corr = ct[f"corr{layer}"]
                    for q in range(4):
                        qs = slice(q * 512, (q + 1) * 512)
                        nc.tensor.matmul(
                            cur[:, qs],
                            lhsT=corr[0:sq_rows, :],
                            rhs=xhsq[layer][:, qs],
                            start=False,
                            stop=True,
                        )

                relu_t[layer] = rp.tile([128, NF], F32, tag=f"relu{layer}", name=f"relu{layer}")
                nc.scalar.activation(
                    out=relu_t[layer][:, :],
                    in_=cur[:, :],
                    func=mybir.ActivationFunctionType.Relu,
                    bias=ct["bias"][:, layer : layer + 1],
                    scale=1.0,
                )

                # direct-output d-reduction
                lo = 64 if layer < 2 else 0
                red_t[layer] = rp.tile([128 - lo, BC], F32, tag=f"red{layer}", name=f"red{layer}")
                nc.vector.tensor_reduce(
                    out=red_t[layer][:, :],
                    in_=relu_t[layer][lo:128, :].rearrange("p (b d) -> p b d", d=D),
                    axis=mybir.AxisListType.X,
                    op=mybir.AluOpType.add,
                )

            nc.sync.dma_start(out=out_d[0:64, :], in_=red_t[0])
            nc.sync.dma_start(out=out_d[64:128, :], in_=red_t[1])
            nc.sync.dma_start(out=out_d[128:256, :], in_=red_t[2])

    nc.compile()
    return nc


def _get_nc():
    if "nc" not in _CACHE:
        _CACHE["nc"] = _build_nc()
    return _CACHE["nc"]


def _install_profile_shim():
    import sys, types

    if "antenv.axon_hooks" in sys.modules:
        return
    try:
        from trn_agent_boot.trn_boot import _ntff_profile_via_ctypes

        hook = _ntff_profile_via_ctypes("/opt/axon/libaxon_pjrt.so")
    except Exception:
        hook = None
    m = types.ModuleType("antenv.axon_hooks")
    m.get_axon_ntff_profile_hook = lambda: hook
    sys.modules["antenv.axon_hooks"] = m


def run(inputs, trace=False, trace_cores=None):
    """Run the SPMD kernel; returns (out (1024,256) fp32, BassKernelResults)."""
    from concourse.bass_utils import run_bass_kernel_spmd

    _install_profile_shim()
    x = np.asarray(inputs["x"], np.float32)
    consts = _host_consts(
        np.asarray(inputs["W0"], np.float32),
        np.asarray(inputs["W1"], np.float32),
        np.asarray(inputs["W2"], np.float32),
    )
    bias = np.stack(
        [np.asarray(inputs[f"b{i}"], np.float32) for i in range(3)], axis=1
    )  # (128, 3)

    in_maps = []
    for c in range(NCORES):
        xT = np.ascontiguousarray(
            x[c * BC : (c + 1) * BC].transpose(1, 0, 2).reshape(39, NF)
        )
        m = {
            "xT": xT,
            "xT3": np.ascontiguousarray(np.tile(xT, (3, 1))),
            "bias": np.ascontiguousarray(bias),
        }
        m.update(consts)
        in_maps.append(m)

    nc = _get_nc()
    res = run_bass_kernel_spmd(
        nc, in_maps, list(range(NCORES)), trace=trace, trace_cores=trace_cores
    )
    out = np.concatenate(
        [res.results[c]["out"].T for c in range(NCORES)], axis=0
    ).astype(np.float32)
    return out, res


def kernel(**inputs):
    out, _ = run(inputs, trace=False)
    return out


# revision 5
# speedup vs baseline: 1.8473x; 1.8473x over previous
"""Trainium2 Bass kernel for nn_CIN (3-layer CIN / xDeepFM feature-interaction).

Reference computation per layer k (x: (B,39,16), h0 = x):
    z[b,f,g,d] = x[b,f,d] * h[b,g,d]
    cur[b,l,d] = relu(sum_{f,g} z[b,f,g,d] * Wk[f*Fk+g, l] + bk[l])
    h <- cur[:, :64] (layers 0,1);  direct outputs concat'd, summed over d.

Sharding: pure data parallelism, batch 1024 -> 8 cores x 128 rows.

Device layout per core: everything is (partition, n) with n = b*16+d in [0,2048).
Per K-chunk of 128 (f,g)-pairs:
  mult path:   PE:  bc = S_c^T @ xT        (broadcast x_f to its (f,g) rows, PSUM)
               DVE: z = bc * h_rep         (outer-product tile, -> SBUF)
  square path: PE:  P = Ssq_c^T @ [xT; h]  (x_f - h_g per row, PSUM)
               ACT: z = Square(P)          (-> SBUF;  x*h = -(1/2)(P^2 - x^2 - h^2),
                                            signs/corrections folded into weights)
  both:        PE:  cur += Wc_c^T @ z      (accumulating matmul, PSUM)
Square-path chunk residuals sum_f .5*W~x[f,l]*x_f^2 + sum_g .5*W~h[g,l]*h_g^2 are
extra small K-chunks with host-folded weights against Square([xT; h]).

The split of chunks between DVE (mult) and ACT (square) balances the two
elementwise engines; TensorE runs both paths' matmuls.
"""

import numpy as np

B, F, D, L = 1024, 39, 16, 128
NCORES = 8
BC = B // NCORES          # 128 batch rows per core
NF = BC * D               # 2048 free elements per core
HALF = NF // 2            # 1024: psum-bank-pair granule
FK = (39, 64, 64)         # h rows per layer
KP = (117, 128, 128)      # chunk partition height per layer
NK = (13, 20, 20)         # chunks per layer
FPER = (3, 2, 2)          # f values per chunk

# Chunks routed to the ACT/square path, per layer (tuned on HW profile).
SQ = (
    frozenset(range(0, 13, 2)),   # 7 of 13
    frozenset(range(0, 20, 2)),   # 10 of 20
    frozenset(range(0, 20, 2)),   # 10 of 20
)

_CACHE = {}


def _round_fp32r(a):
    """Round fp32 -> fp32r (sign + 8 exp + 11 mantissa bits, RNE), bits in fp32."""
    b = np.ascontiguousarray(a, np.float32).view(np.uint32)
    rb = (b >> 12) & 1
    b = (b + np.uint32(0x7FF) + rb) & np.uint32(0xFFFFF000)
    return b.view(np.float32)


def _f_of(layer, ci, p):
    """f,g indices of partition p within chunk ci of a layer; f may be >=39 (pad)."""
    fk = FK[layer]
    return FPER[layer] * ci + p // fk, p % fk


def _host_consts(W0, W1, W2):
    """Fold reference weights into the device constant tensors."""
    Ws = (W0.reshape(39, 39, L), W1.reshape(39, 64, L), W2.reshape(39, 64, L))
    out = {}

    # Broadcast selectors. Smul: (39, nk*kp). Ssq: (39+fk, nk*kp) for sq chunks.
    for layer in (0, 1, 2):
        if layer == 2:
            break  # layers 1,2 share selectors
        lays = (0,) if layer == 0 else (1, 2)
        kp, nk, fk = KP[layer], NK[layer], FK[layer]
        smul = np.zeros((39, nk * kp), np.float32)
        ssq = np.zeros((39 + (0 if layer == 0 else fk), nk * kp), np.float32)
        for ci in range(nk):
            for p in range(kp):
                f, g = _f_of(layer, ci, p)
                if f >= 39:
                    continue
                smul[f, ci * kp + p] = 1.0
                ssq[f, ci * kp + p] = 1.0
                grow = g if layer == 0 else 39 + g
                ssq[grow, ci * kp + p] -= 1.0
        out[f"Smul{layer}"] = smul
        out[f"Ssq{layer}"] = ssq

    # Weight chunks (sq chunks scaled by -1/2) + correction matrices.
    for layer in (0, 1, 2):
        kp, nk, fk = KP[layer], NK[layer], FK[layer]
        W = Ws[layer]
        wc = np.zeros((kp, nk * L), np.float32)
        corr = np.zeros((39 + (fk if layer > 0 else 0), L), np.float32)
        for ci in range(nk):
            sq = ci in SQ[layer]
            for p in range(kp):
                f, g = _f_of(layer, ci, p)
                if f >= 39:
                    continue
                wrow = W[f, g]
                wc[p, ci * L : (ci + 1) * L] = (-0.5 * wrow) if sq else wrow
                if sq:
                    # x*h = .5*x^2 + .5*h^2 - .5*(x-h)^2
                    corr[f] += 0.5 * wrow
                    if layer == 0:
                        corr[g] += 0.5 * wrow
                    else:
                        corr[39 + g] += 0.5 * wrow
        out[f"Wc{layer}"] = wc
        out[f"corr{layer}"] = corr
    return out


def _build_nc():
    import concourse.bacc as bacc
    import concourse.tile as tile
    from concourse import mybir

    F32 = mybir.dt.float32
    F32R = mybir.dt.float32r
    nc = bacc.Bacc("TRN2", target_bir_lowering=False, debug=False, num_devices=NCORES)

    dram = {}

    def din(name, shape, dt=None):
        dram[name] = nc.dram_tensor(
            name, shape, dt or F32R, kind="ExternalInput"
        ).ap()

    din("xT", (39, NF))
    din("xT3", (117, NF))
    din("Smul0", (39, NK[0] * KP[0]))
    din("Smul1", (39, NK[1] * KP[1]))
    din("Ssq0", (39, NK[0] * KP[0]))
    din("Ssq1", (103, NK[1] * KP[1]))
    din("Wc0", (KP[0], NK[0] * L))
    din("Wc1", (KP[1], NK[1] * L))
    din("Wc2", (KP[2], NK[2] * L))
    din("corr0", (39, L))
    din("corr1", (103, L))
    din("corr2", (103, L))
    din("bias", (L, 3), dt=F32)
    out_d = nc.dram_tensor("out", (256, BC), F32, kind="ExternalOutput").ap()

    with tile.TileContext(nc) as tc:
        with (
            tc.tile_pool(name="const", bufs=1) as cp,
            tc.tile_pool(name="work", bufs=2) as wp,
            tc.tile_pool(name="relu", bufs=1) as rp,
            tc.tile_pool(name="zp", bufs=4) as zp,
            tc.tile_pool(name="pbc", bufs=2, space="PSUM") as pbc,
            tc.tile_pool(name="pcur", bufs=1, space="PSUM") as pcur,
        ):
            ct = {}
            for name in dram:
                if name == "out":
                    continue
                shape = list(dram[name].shape)
                ct[name] = cp.tile(shape, dram[name].dtype, tag=name, name=f"c_{name}")
                nc.sync.dma_start(out=ct[name], in_=dram[name])

            relu_t = [None] * 3
            red_t = [None] * 3

            xh = [None] * 3  # square-path rhs per layer ([xT] / [xT; h])
            xhsq = [None] * 3  # Square(xh)
            xh[0] = ct["xT"]

            for layer in (0, 1, 2):
                kp, nk, fk = KP[layer], NK[layer], FK[layer]
                smul = ct["Smul0"] if layer == 0 else ct["Smul1"]
                ssq = ct["Ssq0"] if layer == 0 else ct["Ssq1"]
                wc = ct[f"Wc{layer}"]
                sq_rows = 39 if layer == 0 else 103
                use_sq = len(SQ[layer]) > 0

                # h_rep: (kp, NF) replicated h tile
                if layer == 0:
                    h_rep = ct["xT3"]
                else:
                    prev = relu_t[layer - 1]
                    h_rep = wp.tile([128, NF], F32R, tag="h_rep")
                    nc.sync.dma_start(out=h_rep[0:64, :], in_=prev[0:64, :])
                    nc.sync.dma_start(out=h_rep[64:128, :], in_=prev[0:64, :])
                    if use_sq:
                        xh[layer] = wp.tile([103, NF], F32R, tag="xh", name=f"xh{layer}")
                        nc.sync.dma_start(out=xh[layer][0:39, :], in_=dram["xT"])
                        nc.sync.dma_start(
                            out=xh[layer][39:103, :], in_=prev[0:64, :]
                        )

                if use_sq:
                    xhsq[layer] = wp.tile([sq_rows, NF], F32R, tag="xhsq", name=f"xhsq{layer}")
                    nc.scalar.activation(
                        out=xhsq[layer][:, :],
                        in_=xh[layer][0:sq_rows, :],
                        func=mybir.ActivationFunctionType.Square,
                    )

                cur = pcur.tile([128, NF], F32, tag="cur")
                for ci in range(nk):
                    issq = ci in SQ[layer]
                    for half in range(2):
                        ns = slice(half * HALF, (half + 1) * HALF)
                        bc = pbc.tile([kp, HALF], F32, tag="bc")
                        sel = ssq if issq else smul
                        rhs_src = xh[layer] if issq else ct["xT"]
                        nrows = sq_rows if issq else 39
                        for q in range(2):
                            qs = slice(q * 512, (q + 1) * 512)
                            nqs = slice(half * HALF + q * 512, half * HALF + (q + 1) * 512)
                            nc.tensor.matmul(
                                bc[:, qs],
                                lhsT=sel[0:nrows, ci * kp : (ci + 1) * kp],
                                rhs=rhs_src[0:nrows, nqs],
                                start=True,
                                stop=True,
                            )
                        zt = zp.tile([kp, HALF], F32R, tag="z")
                        if issq:
                            nc.scalar.activation(
                                out=zt[:, :],
                                in_=bc[:, :],
                                func=mybir.ActivationFunctionType.Square,
                            )
                        else:
                            nc.vector.tensor_mul(zt[:, :], bc[:, :], h_rep[0:kp, ns])
                        for q in range(2):
                            qs = slice(q * 512, (q + 1) * 512)
                            nqs = slice(half * HALF + q * 512, half * HALF + (q + 1) * 512)
                            nc.tensor.matmul(
                                cur[:, nqs],
                                lhsT=wc[:, ci * L : (ci + 1) * L],
                                rhs=zt[:, qs],
                                start=(ci == 0),
                                stop=(ci == nk - 1 and not use_sq),
                            )

                if use_sq:
                    corr = ct[f"corr{layer}"]
                    for q in range(4):
                        qs = slice(q * 512, (q + 1) * 512)
                        nc.tensor.matmul(
                            cur[:, qs],
                            lhsT=corr[0:sq_rows, :],
                            rhs=xhsq[layer][:, qs],
                            start=False,
                            stop=True,
                        )

                relu_t[layer] = rp.tile([128, NF], F32R, tag=f"relu{layer}", name=f"relu{layer}")
                nc.scalar.activation(
                    out=relu_t[layer][:, :],
                    in_=cur[:, :],
                    func=mybir.ActivationFunctionType.Relu,
                    bias=ct["bias"][:, layer : layer + 1],
                    scale=1.0,
                )

                # direct-output d-reduction
                lo = 64 if layer < 2 else 0
                red_t[layer] = rp.tile([128 - lo, BC], F32, tag=f"red{layer}", name=f"red{layer}")
                nc.vector.tensor_reduce(
                    out=red_t[layer][:, :],
                    in_=relu_t[layer][lo:128, :].rearrange("p (b d) -> p b d", d=D),
                    axis=mybir.AxisListType.X,
                    op=mybir.AluOpType.add,
                )

            nc.sync.dma_start(out=out_d[0:64, :], in_=red_t[0])
            nc.sync.dma_start(out=out_d[64:128, :], in_=red_t[1])
            nc.sync.dma_start(out=out_d[128:256, :], in_=red_t[2])

    nc.compile()
    return nc


def _get_nc():
    if "nc" not in _CACHE:
        _CACHE["nc"] = _build_nc()
    return _CACHE["nc"]


def _install_profile_shim():
    import sys, types

    if "antenv.axon_hooks" in sys.modules:
        return
    try:
        from trn_agent_boot.trn_boot import _ntff_profile_via_ctypes

        hook = _ntff_profile_via_ctypes("/opt/axon/libaxon_pjrt.so")
    except Exception:
        hook = None
    m = types.ModuleType("antenv.axon_hooks")
    m.get_axon_ntff_profile_hook = lambda: hook
    sys.modules["antenv.axon_hooks"] = m


def run(inputs, trace=False, trace_cores=None):
    """Run the SPMD kernel; returns (out (1024,256) fp32, BassKernelResults)."""
    from concourse.bass_utils import run_bass_kernel_spmd

    _install_profile_shim()
    x = np.asarray(inputs["x"], np.float32)
    consts = _host_consts(
        np.asarray(inputs["W0"], np.float32),
        np.asarray(inputs["W1"], np.float32),
        np.asarray(inputs["W2"], np.float32),
    )
    consts = {k: _round_fp32r(v) for k, v in consts.items()}
    bias = np.stack(
        [np.asarray(inputs[f"b{i}"], np.float32) for i in range(3)], axis=1
    )  # (128, 3)

    in_maps = []
    for c in range(NCORES):
        xT = _round_fp32r(
            x[c * BC : (c + 1) * BC].transpose(1, 0, 2).reshape(39, NF)
        )
        m = {
            "xT": xT,
            "xT3": np.ascontiguousarray(np.tile(xT, (3, 1))),
            "bias": np.ascontiguousarray(bias),
        }
        m.update(consts)
        in_maps.append(m)

    nc = _get_nc()
    res = run_bass_kernel_spmd(
        nc, in_maps, list(range(NCORES)), trace=trace, trace_cores=trace_cores
    )
    out = np.concatenate(
        [res.results[c]["out"].T for c in range(NCORES)], axis=0
    ).astype(np.float32)
    return out, res


def kernel(**inputs):
    out, _ = run(inputs, trace=False)
    return out


# revision 7
# speedup vs baseline: 2.2723x; 1.2301x over previous
"""Trainium2 Bass kernel for nn_CIN (3-layer CIN / xDeepFM feature-interaction).

Reference computation per layer k (x: (B,39,16), h0 = x):
    z[b,f,g,d] = x[b,f,d] * h[b,g,d]
    cur[b,l,d] = relu(sum_{f,g} z[b,f,g,d] * Wk[f*Fk+g, l] + bk[l])
    h <- cur[:, :64] (layers 0,1);  direct outputs concat'd, summed over d.

Sharding: pure data parallelism, batch 1024 -> 8 cores x 128 rows.

Device layout per core: everything is (partition, n) with n = b*16+d in [0,2048).
Per K-chunk of 128 (f,g)-pairs:
  mult path:   PE:  bc = S_c^T @ xT        (broadcast x_f to its (f,g) rows, PSUM)
               DVE: z = bc * h_rep         (outer-product tile, -> SBUF)
  square path: PE:  P = Ssq_c^T @ [xT; h]  (x_f - h_g per row, PSUM)
               ACT: z = Square(P)          (-> SBUF;  x*h = -(1/2)(P^2 - x^2 - h^2),
                                            signs/corrections folded into weights)
  both:        PE:  cur += Wc_c^T @ z      (accumulating matmul, PSUM)
Square-path chunk residuals sum_f .5*W~x[f,l]*x_f^2 + sum_g .5*W~h[g,l]*h_g^2 are
extra small K-chunks with host-folded weights against Square([xT; h]).

The split of chunks between DVE (mult) and ACT (square) balances the two
elementwise engines; TensorE runs both paths' matmuls.
"""

import numpy as np

B, F, D, L = 1024, 39, 16, 128
NCORES = 8
BC = B // NCORES          # 128 batch rows per core
NF = BC * D               # 2048 free elements per core
HALF = NF // 2            # 1024: psum-bank-pair granule
FK = (39, 64, 64)         # h rows per layer
KP = (128, 128, 128)      # chunk partition height per layer (l0: 117 + 11 pad)
NK = (13, 20, 20)         # chunks per layer
FPER = (3, 2, 2)          # f values per chunk

# Chunks routed to the ACT/square path, per layer (tuned on HW profile).
SQ = (
    frozenset(range(0, 13, 2)),   # 7 of 13
    frozenset(range(0, 20, 2)),   # 10 of 20
    frozenset(range(0, 20, 2)),   # 10 of 20
)

_CACHE = {}


def _to_bf16(a):
    import ml_dtypes

    return np.ascontiguousarray(a).astype(ml_dtypes.bfloat16)


def _f_of(layer, ci, p):
    """f,g indices of partition p within chunk ci of a layer; f may be >=39 (pad)."""
    fk = FK[layer]
    if layer == 0 and p >= 117:
        return 39, 0  # pad rows
    return FPER[layer] * ci + p // fk, p % fk


def _host_consts(W0, W1, W2):
    """Fold reference weights into the device constant tensors."""
    Ws = (W0.reshape(39, 39, L), W1.reshape(39, 64, L), W2.reshape(39, 64, L))
    out = {}

    # Broadcast selectors. Smul: (39, nk*kp). Ssq: (39+fk, nk*kp) for sq chunks.
    for layer in (0, 1, 2):
        if layer == 2:
            break  # layers 1,2 share selectors
        lays = (0,) if layer == 0 else (1, 2)
        kp, nk, fk = KP[layer], NK[layer], FK[layer]
        smul = np.zeros((39, nk * kp), np.float32)
        ssq = np.zeros((39 + (0 if layer == 0 else fk), nk * kp), np.float32)
        for ci in range(nk):
            for p in range(kp):
                f, g = _f_of(layer, ci, p)
                if f >= 39:
                    continue
                smul[f, ci * kp + p] = 1.0
                ssq[f, ci * kp + p] = 1.0
                grow = g if layer == 0 else 39 + g
                ssq[grow, ci * kp + p] -= 1.0
        out[f"Smul{layer}"] = smul
        out[f"Ssq{layer}"] = ssq

    # Weight chunks (sq chunks scaled by -1/2) + correction matrices.
    for layer in (0, 1, 2):
        kp, nk, fk = KP[layer], NK[layer], FK[layer]
        W = Ws[layer]
        wc = np.zeros((kp, nk * L), np.float32)
        corr = np.zeros((39 + (fk if layer > 0 else 0), L), np.float32)
        for ci in range(nk):
            sq = ci in SQ[layer]
            for p in range(kp):
                f, g = _f_of(layer, ci, p)
                if f >= 39:
                    continue
                wrow = W[f, g]
                wc[p, ci * L : (ci + 1) * L] = (-0.5 * wrow) if sq else wrow
                if sq:
                    # x*h = .5*x^2 + .5*h^2 - .5*(x-h)^2
                    corr[f] += 0.5 * wrow
                    if layer == 0:
                        corr[g] += 0.5 * wrow
                    else:
                        corr[39 + g] += 0.5 * wrow
        out[f"Wc{layer}"] = wc
        out[f"corr{layer}"] = corr
    return out


def _build_nc():
    import concourse.bacc as bacc
    import concourse.tile as tile
    from concourse import mybir

    F32 = mybir.dt.float32
    BF16 = mybir.dt.bfloat16
    nc = bacc.Bacc("TRN2", target_bir_lowering=False, debug=False, num_devices=NCORES)

    dram = {}

    def din(name, shape, dt=None):
        dram[name] = nc.dram_tensor(
            name, shape, dt or BF16, kind="ExternalInput"
        ).ap()

    din("xT", (39, NF))
    din("xT3", (128, NF))
    din("Smul0", (39, NK[0] * KP[0]))
    din("Smul1", (39, NK[1] * KP[1]))
    din("Ssq0", (39, NK[0] * KP[0]))
    din("Ssq1", (103, NK[1] * KP[1]))
    din("Wc0", (KP[0], NK[0] * L))
    din("Wc1", (KP[1], NK[1] * L))
    din("Wc2", (KP[2], NK[2] * L))
    din("corr0", (39, L))
    din("corr1", (103, L))
    din("corr2", (103, L))
    din("bias", (L, 3), dt=F32)
    out_d = nc.dram_tensor("out", (256, BC), F32, kind="ExternalOutput").ap()

    with tile.TileContext(nc) as tc:
        with (
            tc.tile_pool(name="const", bufs=1) as cp,
            tc.tile_pool(name="work", bufs=2) as wp,
            tc.tile_pool(name="relu", bufs=1) as rp,
            tc.tile_pool(name="zp", bufs=4) as zp,
            tc.tile_pool(name="pbc", bufs=2, space="PSUM") as pbc,
            tc.tile_pool(name="pcur", bufs=1, space="PSUM") as pcur,
        ):
            ct = {}
            for name in dram:
                if name == "out":
                    continue
                shape = list(dram[name].shape)
                ct[name] = cp.tile(shape, dram[name].dtype, tag=name, name=f"c_{name}")
                nc.sync.dma_start(out=ct[name], in_=dram[name])

            relu_t = [None] * 3
            red_t = [None] * 3

            xh = [None] * 3  # square-path rhs per layer ([xT] / [xT; h])
            xhsq = [None] * 3  # Square(xh)
            xh[0] = ct["xT"]

            for layer in (0, 1, 2):
                kp, nk, fk = KP[layer], NK[layer], FK[layer]
                smul = ct["Smul0"] if layer == 0 else ct["Smul1"]
                ssq = ct["Ssq0"] if layer == 0 else ct["Ssq1"]
                wc = ct[f"Wc{layer}"]
                sq_rows = 39 if layer == 0 else 103
                use_sq = len(SQ[layer]) > 0

                # h_rep: (kp, NF) replicated h tile
                if layer == 0:
                    h_rep = ct["xT3"]
                else:
                    prev = relu_t[layer - 1]
                    h_rep = wp.tile([128, NF], BF16, tag="h_rep")
                    nc.sync.dma_start(out=h_rep[0:64, :], in_=prev[0:64, :])
                    nc.sync.dma_start(out=h_rep[64:128, :], in_=prev[0:64, :])
                    if use_sq:
                        xh[layer] = wp.tile([103, NF], BF16, tag="xh", name=f"xh{layer}")
                        nc.sync.dma_start(out=xh[layer][0:39, :], in_=dram["xT"])
                        nc.sync.dma_start(
                            out=xh[layer][39:103, :], in_=prev[0:64, :]
                        )

                if use_sq:
                    xhsq[layer] = wp.tile([sq_rows, NF], BF16, tag="xhsq", name=f"xhsq{layer}")
                    nc.scalar.activation(
                        out=xhsq[layer][:, :],
                        in_=xh[layer][0:sq_rows, :],
                        func=mybir.ActivationFunctionType.Square,
                    )

                cur = pcur.tile([128, NF], F32, tag="cur")
                for ci in range(nk):
                    issq = ci in SQ[layer]
                    for half in range(2):
                        ns = slice(half * HALF, (half + 1) * HALF)
                        bc = pbc.tile([kp, HALF], F32, tag="bc")
                        sel = ssq if issq else smul
                        rhs_src = xh[layer] if issq else ct["xT"]
                        nrows = sq_rows if issq else 39
                        for q in range(2):
                            qs = slice(q * 512, (q + 1) * 512)
                            nqs = slice(half * HALF + q * 512, half * HALF + (q + 1) * 512)
                            nc.tensor.matmul(
                                bc[:, qs],
                                lhsT=sel[0:nrows, ci * kp : (ci + 1) * kp],
                                rhs=rhs_src[0:nrows, nqs],
                                start=True,
                                stop=True,
                            )
                        zt = zp.tile([kp, HALF], BF16, tag="z")
                        if issq:
                            nc.scalar.activation(
                                out=zt[:, :],
                                in_=bc[:, :],
                                func=mybir.ActivationFunctionType.Square,
                            )
                        else:
                            nc.vector.tensor_mul(zt[:, :], bc[:, :], h_rep[0:kp, ns])
                        for q in range(2):
                            qs = slice(q * 512, (q + 1) * 512)
                            nqs = slice(half * HALF + q * 512, half * HALF + (q + 1) * 512)
                            nc.tensor.matmul(
                                cur[:, nqs],
                                lhsT=wc[:, ci * L : (ci + 1) * L],
                                rhs=zt[:, qs],
                                start=(ci == 0),
                                stop=(ci == nk - 1 and not use_sq),
                            )

                if use_sq:
                    corr = ct[f"corr{layer}"]
                    for q in range(4):
                        qs = slice(q * 512, (q + 1) * 512)
                        nc.tensor.matmul(
                            cur[:, qs],
                            lhsT=corr[0:sq_rows, :],
                            rhs=xhsq[layer][:, qs],
                            start=False,
                            stop=True,
                        )

                relu_t[layer] = rp.tile([128, NF], BF16, tag=f"relu{layer}", name=f"relu{layer}")
                nc.scalar.activation(
                    out=relu_t[layer][:, :],
                    in_=cur[:, :],
                    func=mybir.ActivationFunctionType.Relu,
                    bias=ct["bias"][:, layer : layer + 1],
                    scale=1.0,
                )

                # direct-output d-reduction
                lo = 64 if layer < 2 else 0
                red_t[layer] = rp.tile([128 - lo, BC], F32, tag=f"red{layer}", name=f"red{layer}")
                nc.vector.tensor_reduce(
                    out=red_t[layer][:, :],
                    in_=relu_t[layer][lo:128, :].rearrange("p (b d) -> p b d", d=D),
                    axis=mybir.AxisListType.X,
                    op=mybir.AluOpType.add,
                )

            nc.sync.dma_start(out=out_d[0:64, :], in_=red_t[0])
            nc.sync.dma_start(out=out_d[64:128, :], in_=red_t[1])
            nc.sync.dma_start(out=out_d[128:256, :], in_=red_t[2])

    nc.compile()
    return nc


def _get_nc():
    if "nc" not in _CACHE:
        _CACHE["nc"] = _build_nc()
    return _CACHE["nc"]


def _install_profile_shim():
    import sys, types

    if "antenv.axon_hooks" in sys.modules:
        return
    try:
        from trn_agent_boot.trn_boot import _ntff_profile_via_ctypes

        hook = _ntff_profile_via_ctypes("/opt/axon/libaxon_pjrt.so")
    except Exception:
        hook = None
    m = types.ModuleType("antenv.axon_hooks")
    m.get_axon_ntff_profile_hook = lambda: hook
    sys.modules["antenv.axon_hooks"] = m


def host_in_maps(inputs):
    """Host-side sharding + constant folding -> per-core device input maps."""
    x = np.asarray(inputs["x"], np.float32)
    consts = _host_consts(
        np.asarray(inputs["W0"], np.float32),
        np.asarray(inputs["W1"], np.float32),
        np.asarray(inputs["W2"], np.float32),
    )
    consts = {k: _to_bf16(v) for k, v in consts.items()}
    bias = np.stack(
        [np.asarray(inputs[f"b{i}"], np.float32) for i in range(3)], axis=1
    )  # (128, 3)

    in_maps = []
    for c in range(NCORES):
        xT = _to_bf16(
            x[c * BC : (c + 1) * BC].transpose(1, 0, 2).reshape(39, NF)
        )
        m = {
            "xT": xT,
            "xT3": np.ascontiguousarray(np.tile(xT, (4, 1))[:128]),
            "bias": np.ascontiguousarray(bias),
        }
        m.update(consts)
        in_maps.append(m)
    return in_maps


def run(inputs, trace=False, trace_cores=None):
    """Run the SPMD kernel; returns (out (1024,256) fp32, BassKernelResults)."""
    from concourse.bass_utils import run_bass_kernel_spmd

    _install_profile_shim()
    in_maps = host_in_maps(inputs)
    nc = _get_nc()
    res = run_bass_kernel_spmd(
        nc, in_maps, list(range(NCORES)), trace=trace, trace_cores=trace_cores
    )
    out = np.concatenate(
        [res.results[c]["out"].T for c in range(NCORES)], axis=0
    ).astype(np.float32)
    return out, res


def kernel(**inputs):
    out, _ = run(inputs, trace=False)
    return out


# revision 8
# speedup vs baseline: 2.5432x; 1.1192x over previous
"""Trainium2 Bass kernel for nn_CIN (3-layer CIN / xDeepFM feature-interaction).

Reference computation per layer k (x: (B,39,16), h0 = x):
    z[b,f,g,d] = x[b,f,d] * h[b,g,d]
    cur[b,l,d] = relu(sum_{f,g} z[b,f,g,d] * Wk[f*Fk+g, l] + bk[l])
    h <- cur[:, :64] (layers 0,1);  direct outputs concat'd, summed over d.

Sharding: pure data parallelism, batch 1024 -> 8 cores x 128 rows.

Device layout per core: everything is (partition, n) with n = b*16+d in [0,2048).
The (f,g) interaction pairs are covered by K-chunks of 128 pairs, each either:
  mult path:   PE:  bc = S_c^T @ x-tile    (broadcast x_f to its (f,g) rows, PSUM)
               DVE: z = bc * h_rep        (outer-product tile, -> SBUF bf16)
  square path: PE:  P = Ssq_c^T @ [x; h]  (x_f - h_g per row, PSUM)
               ACT: z = Square(P)         (x*h = -(1/2)((x-h)^2 - x^2 - h^2);
                                           signs/corrections folded into weights)
  both:        PE:  cur += Wc_c^T @ z     (accumulating matmul, PSUM)
Square-path residuals (sum of .5*w*x_f^2 + .5*w*h_g^2) are one extra K-chunk per
layer with host-folded weights against Square([x; h]).

Layer 0 exploits x (x) x symmetry: f<SPLIT0 rows go through the mult path with
original ordered weights; all remaining ordered pairs are folded onto unordered
pairs (w = W[a,b]+W[b,a], or the single missing order) handled by the square
path, whose selector columns are unconstrained. Diagonal terms fold into the
correction chunk. This cuts layer-0 chunks 13 -> 10.

Layer 1/2 mult-path broadcasts are 2-way row-group packed: f-groups 0..19 and
20..38 live at partition bases 0 and 64 of one x tile, so consecutive broadcast
matmuls hit disjoint PE row quadrants and run concurrently.

All matmul operands are bf16 (fp32 PSUM accumulate); fp32 elsewhere.
"""

import numpy as np

B, F, D, L = 1024, 39, 16, 128
NCORES = 8
BC = B // NCORES          # 128 batch rows per core
NF = BC * D               # 2048 free elements per core
HALF = NF // 2            # 1024: psum-bank-pair granule
KP = 128                  # chunk height (partitions)
SPLIT0 = 15               # layer-0: f < SPLIT0 handled by mult path
N_WARM = 28               # PE warm-up matmuls at kernel start

_CACHE = {}


def _plan():
    """Structural chunk plan (no weight values), shared by host + device.

    Returns per-layer list of chunk dicts:
      mult: {kind:'mult', fbase, nf, group}   (l0: f-triples; l1/2: f-pairs)
      sq:   {kind:'sq', pairs: [(urow, vrow, f, g, mode)]}
          urow/vrow: row indices in the bcast rhs tile (l0: xT; l1/2: xh)
          (f, g, mode): which W entries fold into this pair's weight:
             mode 'one'  -> W[f,g]
             mode 'sym'  -> W[f,g] + W[g,f]
    """
    layers = []
    # ---- layer 0 ----
    mult = [{"kind": "mult", "fbase": 3 * i, "nf": 3, "group": 0} for i in range(5)]
    entries = []
    for a in range(SPLIT0):
        for b in range(SPLIT0, 39):
            entries.append((b, a, b, a, "one"))       # missing order (f=b, g=a)
    for a in range(SPLIT0, 39):
        for b in range(a + 1, 39):
            entries.append((a, b, a, b, "sym"))
    sq = []
    for i in range(0, len(entries), KP):
        sq.append({"kind": "sq", "pairs": entries[i : i + KP]})
    order = []
    mi, si = 0, 0
    while mi < len(mult) or si < len(sq):
        if mi < len(mult):
            order.append(mult[mi]); mi += 1
        if si < len(sq):
            order.append(sq[si]); si += 1
    layers.append(order)
    # ---- layers 1, 2 (identical structure) ----
    fpairs = [(2 * i, 0) for i in range(10)] + [(20 + 2 * i, 1) for i in range(10)]
    mult, sq = [], []
    for idx, (fb, grp) in enumerate(fpairs):
        nf = 1 if fb == 38 else 2
        if idx % 2 == 1:
            sq.append({
                "kind": "sq",
                "pairs": [
                    (f, 39 + g, f, g, "one")
                    for f in range(fb, min(fb + nf, 39))
                    for g in range(64)
                ],
            })
        else:
            mult.append({"kind": "mult", "fbase": fb, "nf": nf, "group": grp})
    # order: mult-g0, mult-g1, sq, sq (keeps packed bcast pairs adjacent)
    mg0 = [c for c in mult if c["group"] == 0]
    mg1 = [c for c in mult if c["group"] == 1]
    order = []
    while mg0 or mg1 or sq:
        if mg0:
            order.append(mg0.pop(0))
        if mg1:
            order.append(mg1.pop(0))
        if sq:
            order.append(sq.pop(0))
        if sq:
            order.append(sq.pop(0))
    layers.append(order)
    layers.append(order)  # same structure for layer 2
    return layers


PLAN = _plan()
NCH = [len(p) for p in PLAN]


def _mult_rows(layer, c):
    """(tile_row, f, g) triples for a mult chunk's 128 z-rows (f>=39 = pad)."""
    fk = 39 if layer == 0 else 64
    out = []
    for p in range(KP):
        f = c["fbase"] + p // fk
        if p // fk >= c["nf"] or f >= 39:
            out.append((p, 39, 0))
        else:
            out.append((p, f, p % fk))
    return out


def _host_consts(W0, W1, W2):
    """Fold reference weights into device constant tensors (fp32, cast later)."""
    Ws = (W0.reshape(39, 39, L), W1.reshape(39, 64, L), W2.reshape(39, 64, L))
    out = {}
    corr_all = np.zeros((103, 3 * L), np.float32)
    for layer in (0, 1, 2):
        W = Ws[layer]
        nch = NCH[layer]
        wc = np.zeros((KP, nch * L), np.float32)
        corr = corr_all[:, layer * L : (layer + 1) * L]
        if layer == 0:
            for a in range(SPLIT0, 39):     # diagonal x_a^2 terms, a >= SPLIT0
                corr[a] += W[a, a]
        if layer < 2:
            smul_rows = 39 if layer == 0 else 84
            ssq_rows = 39 if layer == 0 else 103
            smul = np.zeros((smul_rows, nch * KP), np.float32)
            ssq = np.zeros((ssq_rows, nch * KP), np.float32)
        for ci, c in enumerate(PLAN[layer]):
            if c["kind"] == "mult":
                for p, f, g in _mult_rows(layer, c):
                    if f >= 39:
                        continue
                    wc[p, ci * L : (ci + 1) * L] = W[f, g]
                    if layer < 2:
                        srow = f if layer == 0 else (f if c["group"] == 0 else 64 + f - 20)
                        smul[srow, ci * KP + p] = 1.0
            else:
                for p, (ur, vr, f, g, mode) in enumerate(c["pairs"]):
                    w = W[f, g] + (W[g, f] if mode == "sym" else 0.0)
                    wc[p, ci * L : (ci + 1) * L] = -0.5 * w
                    corr[ur] += 0.5 * w
                    corr[vr] += 0.5 * w
                    if layer < 2:
                        ssq[ur, ci * KP + p] += 1.0
                        ssq[vr, ci * KP + p] -= 1.0
        out[f"Wc{layer}"] = wc
        if layer < 2:
            out[f"Smul{layer}"] = smul
            out[f"Ssq{layer}"] = ssq
    out["corr"] = corr_all
    return out


def _build_nc():
    import concourse.bacc as bacc
    import concourse.tile as tile
    from concourse import mybir

    F32 = mybir.dt.float32
    BF16 = mybir.dt.bfloat16
    nc = bacc.Bacc("TRN2", target_bir_lowering=False, debug=False, num_devices=NCORES)

    dram = {}

    def din(name, shape, dt=BF16):
        dram[name] = nc.dram_tensor(name, shape, dt, kind="ExternalInput").ap()

    # declaration order == DMA issue order: layer-0 critical first
    din("xT", (39, NF))
    din("Smul0", (39, NCH[0] * KP))
    din("Ssq0", (39, NCH[0] * KP))
    din("Wc0", (KP, NCH[0] * L))
    din("corr", (103, 3 * L))
    din("bias", (L, 3), dt=F32)
    din("xT3", (KP, NF))
    din("xT2", (84, NF))
    din("Smul1", (84, NCH[1] * KP))
    din("Ssq1", (103, NCH[1] * KP))
    din("Wc1", (KP, NCH[1] * L))
    din("Wc2", (KP, NCH[2] * L))
    out_d = nc.dram_tensor("out", (256, BC), F32, kind="ExternalOutput").ap()

    with tile.TileContext(nc) as tc:
        with (
            tc.tile_pool(name="const", bufs=1) as cp,
            tc.tile_pool(name="work", bufs=2) as wp,
            tc.tile_pool(name="relu", bufs=1) as rp,
            tc.tile_pool(name="zp", bufs=6) as zp,
            tc.tile_pool(name="pbc", bufs=2, space="PSUM") as pbc,
            tc.tile_pool(name="pcur", bufs=1, space="PSUM") as pcur,
        ):
            ct = {}
            for name in dram:
                if name == "out":
                    continue
                ct[name] = cp.tile(
                    list(dram[name].shape), dram[name].dtype, tag=name, name=f"c_{name}"
                )
                nc.sync.dma_start(out=ct[name], in_=dram[name])

            # PE warm-up: dummy matmuls into a rotating bc-psum slot while
            # the remaining constants stream in (HAM needs ~3.4us busy).
            for wi in range(N_WARM):
                wt = pbc.tile([KP, HALF], F32, tag="bc", name=f"warm{wi}")
                nc.tensor.matmul(
                    wt[:, 0:512],
                    lhsT=ct["Smul0"][0:39, 0:KP],
                    rhs=ct["xT"][0:39, 0:512],
                    start=True,
                    stop=True,
                )

            relu_h = [None] * 3   # rows 0:64 (next-layer h), bf16
            relu_d = [None] * 3   # direct rows, bf16
            red_t = [None] * 3

            xh = [None] * 3
            xhsq = [None] * 3
            xh[0] = ct["xT"]

            for layer in (0, 1, 2):
                wc = ct[f"Wc{layer}"]
                sq_rows = 39 if layer == 0 else 103
                smul = ct["Smul0"] if layer == 0 else ct["Smul1"]
                ssq = ct["Ssq0"] if layer == 0 else ct["Ssq1"]

                if layer == 0:
                    h_rep = ct["xT3"]
                else:
                    prev = relu_h[layer - 1]
                    h_rep = wp.tile([128, NF], BF16, tag="h_rep")
                    nc.gpsimd.dma_start(out=h_rep[0:64, :], in_=prev[0:64, :])
                    nc.gpsimd.dma_start(out=h_rep[64:128, :], in_=prev[0:64, :])
                    xh[layer] = wp.tile([103, NF], BF16, tag="xh", name=f"xh{layer}")
                    nc.gpsimd.dma_start(out=xh[layer][0:39, :], in_=dram["xT"])
                    nc.gpsimd.dma_start(out=xh[layer][39:103, :], in_=prev[0:64, :])

                xhsq[layer] = wp.tile(
                    [sq_rows, NF], BF16, tag="xhsq", name=f"xhsq{layer}"
                )
                nc.scalar.activation(
                    out=xhsq[layer][:, :],
                    in_=xh[layer][0:sq_rows, :],
                    func=mybir.ActivationFunctionType.Square,
                )

                cur = pcur.tile([128, NF], F32, tag="cur")
                for ci, c in enumerate(PLAN[layer]):
                    issq = c["kind"] == "sq"
                    if issq:
                        sel, rlo, rhi = ssq, 0, sq_rows
                        rhs_src = xh[layer]
                    elif layer == 0 or c["group"] == 0:
                        sel, rlo, rhi = smul, 0, (39 if layer == 0 else 20)
                        rhs_src = ct["xT"] if layer == 0 else ct["xT2"]
                    else:
                        sel, rlo, rhi = smul, 64, 84
                        rhs_src = ct["xT2"]
                    for half in range(2):
                        ns = slice(half * HALF, (half + 1) * HALF)
                        bc = pbc.tile([KP, HALF], F32, tag="bc")
                        for q in range(2):
                            qs = slice(q * 512, (q + 1) * 512)
                            nqs = slice(
                                half * HALF + q * 512, half * HALF + (q + 1) * 512
                            )
                            nc.tensor.matmul(
                                bc[:, qs],
                                lhsT=sel[rlo:rhi, ci * KP : (ci + 1) * KP],
                                rhs=rhs_src[rlo:rhi, nqs],
                                start=True,
                                stop=True,
                            )
                        zt = zp.tile([KP, HALF], BF16, tag="z")
                        if issq:
                            nc.scalar.activation(
                                out=zt[:, :],
                                in_=bc[:, :],
                                func=mybir.ActivationFunctionType.Square,
                            )
                        else:
                            nc.vector.tensor_mul(zt[:, :], bc[:, :], h_rep[0:KP, ns])
                        for q in range(2):
                            qs = slice(q * 512, (q + 1) * 512)
                            nqs = slice(
                                half * HALF + q * 512, half * HALF + (q + 1) * 512
                            )
                            nc.tensor.matmul(
                                cur[:, nqs],
                                lhsT=wc[:, ci * L : (ci + 1) * L],
                                rhs=zt[:, qs],
                                start=(ci == 0),
                                stop=False,
                            )

                # correction chunk (always last accumulation into each bank)
                corr = ct["corr"][0:sq_rows, layer * L : (layer + 1) * L]
                for q in range(4):
                    qs = slice(q * 512, (q + 1) * 512)
                    nc.tensor.matmul(
                        cur[:, qs],
                        lhsT=corr,
                        rhs=xhsq[layer][:, qs],
                        start=False,
                        stop=True,
                    )

                # relu: h-half first (critical path), direct half after
                bias_ap = ct["bias"][:, layer : layer + 1]
                if layer < 2:
                    relu_h[layer] = rp.tile(
                        [64, NF], BF16, tag=f"rh{layer}", name=f"rh{layer}"
                    )
                    nc.scalar.activation(
                        out=relu_h[layer][:, :],
                        in_=cur[0:64, :],
                        func=mybir.ActivationFunctionType.Relu,
                        bias=bias_ap[0:64],
                        scale=1.0,
                    )
                    relu_d[layer] = rp.tile(
                        [64, NF], BF16, tag=f"rd{layer}", name=f"rd{layer}"
                    )
                    nc.scalar.activation(
                        out=relu_d[layer][:, :],
                        in_=cur[64:128, :],
                        func=mybir.ActivationFunctionType.Relu,
                        bias=bias_ap[64:128],
                        scale=1.0,
                    )
                else:
                    relu_d[layer] = rp.tile(
                        [128, NF], BF16, tag=f"rd{layer}", name=f"rd{layer}"
                    )
                    nc.scalar.activation(
                        out=relu_d[layer][:, :],
                        in_=cur[:, :],
                        func=mybir.ActivationFunctionType.Relu,
                        bias=bias_ap,
                        scale=1.0,
                    )

                nr = 64 if layer < 2 else 128
                red_t[layer] = rp.tile([nr, BC], F32, tag=f"red{layer}", name=f"red{layer}")
                nc.vector.tensor_reduce(
                    out=red_t[layer][:, :],
                    in_=relu_d[layer].rearrange("p (b d) -> p b d", d=D),
                    axis=mybir.AxisListType.X,
                    op=mybir.AluOpType.add,
                )

            nc.sync.dma_start(out=out_d[0:64, :], in_=red_t[0])
            nc.sync.dma_start(out=out_d[64:128, :], in_=red_t[1])
            nc.sync.dma_start(out=out_d[128:256, :], in_=red_t[2])

    nc.compile()
    return nc


def _get_nc():
    if "nc" not in _CACHE:
        _CACHE["nc"] = _build_nc()
    return _CACHE["nc"]


def _install_profile_shim():
    import sys, types

    if "antenv.axon_hooks" in sys.modules:
        return
    try:
        from trn_agent_boot.trn_boot import _ntff_profile_via_ctypes

        hook = _ntff_profile_via_ctypes("/opt/axon/libaxon_pjrt.so")
    except Exception:
        hook = None
    m = types.ModuleType("antenv.axon_hooks")
    m.get_axon_ntff_profile_hook = lambda: hook
    sys.modules["antenv.axon_hooks"] = m


def _to_bf16(a):
    import ml_dtypes

    return np.ascontiguousarray(a).astype(ml_dtypes.bfloat16)


def host_in_maps(inputs):
    """Host-side sharding + constant folding -> per-core device input maps."""
    x = np.asarray(inputs["x"], np.float32)
    consts = _host_consts(
        np.asarray(inputs["W0"], np.float32),
        np.asarray(inputs["W1"], np.float32),
        np.asarray(inputs["W2"], np.float32),
    )
    consts = {k: _to_bf16(v) for k, v in consts.items()}
    bias = np.stack(
        [np.asarray(inputs[f"b{i}"], np.float32) for i in range(3)], axis=1
    )  # (128, 3)

    in_maps = []
    for c in range(NCORES):
        xT = _to_bf16(x[c * BC : (c + 1) * BC].transpose(1, 0, 2).reshape(39, NF))
        xT2 = np.zeros((84, NF), xT.dtype)
        xT2[0:20] = xT[0:20]
        xT2[64:83] = xT[20:39]
        m = {
            "xT": xT,
            "xT3": np.ascontiguousarray(np.tile(xT, (4, 1))[:KP]),
            "xT2": xT2,
            "bias": np.ascontiguousarray(bias),
        }
        m.update(consts)
        in_maps.append(m)
    return in_maps


def run(inputs, trace=False, trace_cores=None):
    """Run the SPMD kernel; returns (out (1024,256) fp32, BassKernelResults)."""
    from concourse.bass_utils import run_bass_kernel_spmd

    _install_profile_shim()
    in_maps = host_in_maps(inputs)
    nc = _get_nc()
    res = run_bass_kernel_spmd(
        nc, in_maps, list(range(NCORES)), trace=trace, trace_cores=trace_cores
    )
    out = np.concatenate(
        [res.results[c]["out"].T for c in range(NCORES)], axis=0
    ).astype(np.float32)
    return out, res


def kernel(**inputs):
    out, _ = run(inputs, trace=False)
    return out
